# revision 1
# baseline (speedup 1.0000x reference)
"""ContrastiveLoss Trainium2 kernel — adjacency-paired gathers.

Same data-parallel layout as kernel.py (8 cores = 4 batches x 2 halves,
27500 sample pairs per core), same indirect1d gather primitive (one index
per SBUF partition per instruction, streaming the out free dim contiguously
from that row — hardware-verified semantics).

New: the per-instruction cost (~1.4 us) is independent of the streamed
length, so samples whose A-rows (or B-rows) lie at DRAM distance 1 or 2 are
host-paired and fetched by one descriptor streaming gap+1 rows (interior
cells wasted). Greedy ROUNDS — gaps 1..16, alternating A then B, each on the
samples earlier rounds left unused — remove ~74 of the 432 naive gather
instructions (357 remain).

Slot layout per core (COLS columns x 128 partitions): one region per
(side, gap) round, block width gap+1, then [singles + pads].
A gap-g pair instruction k reads ia[:, c0] and writes columns c0..c0+g (the
host guarantees ia[p, c0+g] == ia[p, c0]+g); the partner side uses normal
C=1 columns at the real cells, and the wasted interior cells of gap>=2 blocks
are zeroed by strided memsets (weights there are 0; the memset keeps
stale SBUF NaN/Inf out of the weighted sum).
Per-slot f32 weights wm/wn (1.0 for match/nonmatch, 0 for pads) replace the
old block masks: dist -> match partial = sum dist*wm, nonmatch partial =
sum relu(0.5-dist)*wn, partition-reduced by a ones-vector matmul.
"""

import os

import numpy as np

B, N, D = 4, 307200, 16
M_MATCH, M_NONMATCH = 5000, 50000
MARGIN = 0.5
NON_MATCH_WEIGHT = 1.0
NCORES = 8

P = 128
MH = M_MATCH // 2
NH = M_NONMATCH // 2
NS = MH + NH  # 27500 samples per core
CHUNK = 32  # compute-chunk width in columns (straddling pair blocks are
# safe: the tile framework tracks dependencies per byte range)

LAST_EXEC_NS = None
_CACHE = {}


def _pair_scan(vals, order, gap):
    """Greedy pairing over `order` (sorted by vals): pair consecutive sorted
    entries whose values differ by exactly `gap`."""
    pairs = []
    i = 0
    while i < len(order) - 1:
        s0, s1 = order[i], order[i + 1]
        if vals[s1] == vals[s0] + gap:
            pairs.append((s0, s1))
            i += 2
        else:
            i += 1
    return pairs


# pairing rounds: (side, gap); side 0 pairs on A-rows, side 1 on B-rows.
# Each round runs greedily on the samples earlier rounds left unused.
ROUNDS = [(side, gap) for gap in range(1, 17) for side in (0, 1)]


def _plan(a, b):
    used = np.zeros(len(a), np.bool_)
    out = []
    for side, gap in ROUNDS:
        vals = a if side == 0 else b
        rem = np.where(~used)[0]
        order = rem[np.argsort(vals[rem], kind="stable")]
        pairs = _pair_scan(vals, order, gap)
        for s0, s1 in pairs:
            used[s0] = used[s1] = True
        out.append(pairs)
    return out


def _build_nc(ns, COLS):
    import concourse.bacc as bacc
    import concourse.mybir as mybir
    import concourse.tile as tile
    from concourse import bass

    f32 = mybir.dt.float32
    i32 = mybir.dt.int32
    X = mybir.AxisListType.X
    ADD = mybir.AluOpType.add
    MULT = mybir.AluOpType.mult
    Relu = mybir.ActivationFunctionType.Relu

    nc = bacc.Bacc(
        "TRN2", target_bir_lowering=False, debug=False, num_swdge_queues=2
    )
    eA = nc.dram_tensor("eA", (N, D), f32, kind="ExternalInput")
    eB = nc.dram_tensor("eB", (N, D), f32, kind="ExternalInput")
    ia = nc.dram_tensor("ia", (P, COLS), i32, kind="ExternalInput")
    ib = nc.dram_tensor("ib", (P, COLS), i32, kind="ExternalInput")
    wm = nc.dram_tensor("wm", (P, COLS), f32, kind="ExternalInput")
    wn = nc.dram_tensor("wn", (P, COLS), f32, kind="ExternalInput")
    out = nc.dram_tensor("out", (1, 2), f32, kind="ExternalOutput")

    qctr = [0]

    def gather(dst_ap, src, idx_ap):
        inst = nc.gpsimd.indirect_dma_start(
            out=dst_ap,
            out_offset=None,
            in_=src.ap(),
            in_offset=bass.IndirectOffsetOnAxis(ap=idx_ap, axis=0),
        )
        if qctr[0] % 2:
            inst.ins.queue = "qPoolDynamic1"
        qctr[0] += 1

    with tile.TileContext(nc) as tc:
        with (
            tc.tile_pool(name="io", bufs=1) as iop,
            tc.tile_pool(name="gath", bufs=1) as gp,
            tc.tile_pool(name="cmp", bufs=4) as cp,
            tc.tile_pool(name="psum", bufs=1, space="PSUM") as pp,
        ):
            # first compute chunk's indices load first
            c0 = min(CHUNK, COLS)
            ia_t = iop.tile([P, COLS], i32)
            nc.sync.dma_start(ia_t[:, :c0], ia.ap()[:, :c0])
            ib_t = iop.tile([P, COLS], i32)
            nc.sync.dma_start(ib_t[:, :c0], ib.ap()[:, :c0])
            if COLS > c0:
                nc.sync.dma_start(ia_t[:, c0:], ia.ap()[:, c0:])
                nc.sync.dma_start(ib_t[:, c0:], ib.ap()[:, c0:])
            wm_t = iop.tile([P, COLS], f32)
            nc.sync.dma_start(wm_t[:], wm.ap())
            wn_t = iop.tile([P, COLS], f32)
            nc.sync.dma_start(wn_t[:], wn.ap())
            margin_t = iop.tile([P, 1], f32)
            nc.vector.memset(margin_t[:], MARGIN)

            gA = gp.tile([P, COLS * D], f32)
            gB = gp.tile([P, COLS * D], f32)
            dist = gp.tile([P, COLS], f32)
            hng = gp.tile([P, COLS], f32)

            regions = []  # (start, end, side, gap) in ROUNDS order
            base = 0
            for (side, gap), n in zip(ROUNDS, ns):
                w = gap + 1
                regions.append((base, base + w * n, side, gap))
                base += w * n

            # gap>=2 pair blocks leave interior cells unwritten on the
            # single-descriptor side; zero them so stale SBUF can't poison
            # the (weight-0) distance with NaN/Inf.
            for start, end, side, gap in regions:
                if gap >= 2 and end > start:
                    buf = gB if side == 0 else gA
                    nc.vector.memset(
                        buf[:, start * D : end * D].rearrange(
                            "p (m c) -> p m c", c=(gap + 1) * D
                        )[:, :, D : gap * D],
                        0.0,
                    )

            def emit_col(c):
                for start, end, side, gap in regions:
                    if c < end:
                        loc = (c - start) % (gap + 1)
                        pair_src = (gA, eA, ia_t) if side == 0 else (gB, eB, ib_t)
                        sgl_src = (gB, eB, ib_t) if side == 0 else (gA, eA, ia_t)
                        if loc == 0:
                            g_t, e_t, i_t = pair_src
                            gather(
                                g_t[:, c * D : (c + gap + 1) * D],
                                e_t,
                                i_t[:, c : c + 1],
                            )
                        if loc == 0 or loc == gap:
                            g_t, e_t, i_t = sgl_src
                            gather(
                                g_t[:, c * D : (c + 1) * D], e_t, i_t[:, c : c + 1]
                            )
                        return
                gather(gA[:, c * D : (c + 1) * D], eA, ia_t[:, c : c + 1])
                gather(gB[:, c * D : (c + 1) * D], eB, ib_t[:, c : c + 1])

            for cs in range(0, COLS, CHUNK):
                ce = min(cs + CHUNK, COLS)
                for c in range(cs, ce):
                    emit_col(c)
                w = ce - cs
                nd = cp.tile([P, CHUNK * D], f32, tag="nd")
                nc.vector.tensor_sub(
                    nd[:, : w * D], gA[:, cs * D : ce * D], gB[:, cs * D : ce * D]
                )
                nsq = cp.tile([P, CHUNK * D], f32, tag="nsq")
                nc.scalar.square(nsq[:, : w * D], nd[:, : w * D])
                nc.vector.tensor_reduce(
                    dist[:, cs:ce],
                    nsq[:, : w * D].rearrange("p (s d) -> p s d", d=D),
                    axis=X,
                    op=ADD,
                )
                nc.scalar.activation(
                    hng[:, cs:ce],
                    dist[:, cs:ce],
                    Relu,
                    bias=margin_t[:],
                    scale=-1.0,
                )

            # weighted partials + cross-partition reduction
            sums = gp.tile([P, 2], f32)
            md = cp.tile([P, COLS], f32, tag="md")
            nc.vector.tensor_tensor(out=md[:], in0=dist[:], in1=wm_t[:], op=MULT)
            nc.vector.tensor_reduce(sums[:, 0:1], md[:], axis=X, op=ADD)
            nh = cp.tile([P, COLS], f32, tag="nh")
            nc.vector.tensor_tensor(out=nh[:], in0=hng[:], in1=wn_t[:], op=MULT)
            nc.vector.tensor_reduce(sums[:, 1:2], nh[:], axis=X, op=ADD)

            ones = gp.tile([P, 1], f32)
            nc.vector.memset(ones[:], 1.0)
            acc = pp.tile([1, 2], f32, space="PSUM")
            nc.tensor.matmul(acc[:], lhsT=ones[:], rhs=sums[:], start=True, stop=True)
            res = gp.tile([1, 2], f32)
            nc.vector.tensor_copy(res[:], acc[:])
            nc.sync.dma_start(out.ap(), res[:])

    nc.compile()
    return nc


def _in_maps(outA, outB, matchA, matchB, nonMatchA, nonMatchB):
    outA = np.ascontiguousarray(np.asarray(outA, dtype=np.float32))
    outB = np.ascontiguousarray(np.asarray(outB, dtype=np.float32))
    matchA = np.asarray(matchA).astype(np.int64)
    matchB = np.asarray(matchB).astype(np.int64)
    nonMatchA = np.asarray(nonMatchA).astype(np.int64)
    nonMatchB = np.asarray(nonMatchB).astype(np.int64)

    cores = []
    for c in range(NCORES):
        b, h = c // 2, c % 2
        a = np.concatenate(
            [matchA[b, h * MH : (h + 1) * MH], nonMatchA[b, h * NH : (h + 1) * NH]]
        )
        bb = np.concatenate(
            [matchB[b, h * MH : (h + 1) * MH], nonMatchB[b, h * NH : (h + 1) * NH]]
        )
        ismatch = np.zeros(NS, np.bool_)
        ismatch[:MH] = True
        plists = _plan(a, bb)
        cores.append((a, bb, ismatch, plists))

    # shared kernel shape: full pair instructions only, min across cores
    ns = [
        min(len(core[3][r]) for core in cores) // P for r in range(len(ROUNDS))
    ]
    n_in_pairs = 2 * P * sum(ns)
    nScols = -(-(NS - n_in_pairs) // P)  # leftover pairs spill into singles
    COLS = sum(n * (gap + 1) for n, (_, gap) in zip(ns, ROUNDS)) + nScols

    maps = []
    for ci, (a, bb, ismatch, plists) in enumerate(cores):
        b = ci // 2
        ia = np.zeros((P, COLS), np.int32)
        ib = np.zeros((P, COLS), np.int32)
        wm = np.zeros((P, COLS), np.float32)
        wn = np.zeros((P, COLS), np.float32)
        used = np.zeros(NS, np.bool_)

        def place(s, p, col):
            ia[p, col] = a[s]
            ib[p, col] = bb[s]
            wm[p, col] = 1.0 if ismatch[s] else 0.0
            wn[p, col] = 0.0 if ismatch[s] else 1.0
            used[s] = True

        base = 0
        for r, ((side, gap), n) in enumerate(zip(ROUNDS, ns)):
            w = gap + 1
            pl = plists[r]
            for t in range(n * P):
                s0, s1 = pl[t]
                k, p = divmod(t, P)
                place(s0, p, base + w * k)
                place(s1, p, base + w * k + gap)
            iv = ia if side == 0 else ib
            for k in range(n):
                cc = base + w * k
                assert np.all(iv[:, cc + gap] == iv[:, cc] + gap)
            base += w * n
        singles = np.where(~used)[0]
        for i, s in enumerate(singles):
            place(s, i % P, base + i // P)

        maps.append(
            {
                "eA": outA[b],
                "eB": outB[b],
                "ia": ia,
                "ib": ib,
                "wm": wm,
                "wn": wn,
            }
        )
    return maps, ns, COLS


def kernel(outA, outB, matchA, matchB, nonMatchA, nonMatchB):
    global LAST_EXEC_NS
    from concourse import bass_utils

    maps, ns, COLS = _in_maps(
        outA, outB, matchA, matchB, nonMatchA, nonMatchB
    )
    ck = (tuple(ns), COLS)
    if _CACHE.get("key") != ck:
        _CACHE["nc"] = _build_nc(ns, COLS)
        _CACHE["key"] = ck
    nc = _CACHE["nc"]

    kwargs = {}
    if os.environ.get("KERNEL_TRACE", "0") == "1":
        kwargs["trace"] = True
    r = bass_utils.run_bass_kernel_spmd(
        nc, maps, core_ids=list(range(NCORES)), **kwargs
    )
    LAST_EXEC_NS = r.exec_time_ns

    partial = np.stack(
        [np.asarray(r.results[c]["out"]).ravel() for c in range(NCORES)]
    )
    match_loss = partial[:, 0].sum(dtype=np.float64) / M_MATCH
    nonmatch_loss = (
        NON_MATCH_WEIGHT * partial[:, 1].sum(dtype=np.float64) / M_NONMATCH
    )
    contrastive = match_loss + nonmatch_loss
    return (
        np.float32(contrastive),
        np.float32(match_loss),
        np.float32(nonmatch_loss),
    )

